# revision 31
# baseline (speedup 1.0000x reference)
"""BiLSTM-CRF loss kernel for 8 Trainium2 NeuronCores.

Math (per sequence):
  NLL = log Z - gold
  log Z:  forward algorithm over L=1024 steps, T=32 tags.
  gold:   score of the labelled path.

Segmented rank-1 skeleton algorithm (device):
  Z = stop^T M_{L-1}...M_0 e_START with M_l = D_l X, X = exp(trans),
  D_l = diag(exp(f_l - mu)).  L is split into S=32 segments of K=32
  steps.  Products of >=16 consecutive M_l are numerically rank-1
  (Birkhoff contraction; the diagonals cancel in cross-ratios), so
  P_s ~= (P_s p)(q^T P_s)/(q^T P_s p).  Each segment's forward probe
  a_s = P_s p and backward probe b_s = P_s^T q are independent vector
  recurrences of K steps, so the serial depth is K=32 supersteps
  instead of 512:

    log Z = L*mu + sum_{s=1}^{S-1} log(b_s . a_{s-1})
                 - sum_{s=1}^{S-2} log(sum a_s)

  with p = e_START for s=0 and q = X[STOP,:] for s=S-1 (both exact).

  gold is not run through the recurrence at all:
    emit  = sum_l feats[b,l,tags[b,l]]      (host-gathered values,
                                             device reduction)
    trans = sum_{pairs} count[b,pair] * trans[pair]
                                            (host-counted pairs,
                                             device matmuls)

Layout per core (128 sequences): 64 chains (fwd+bwd per segment) ride
the 128 partitions as 8 "quads" of 4 segments x 32 tags; columns are
sequences.  exp(f - mu) runs once per element on the Activation engine
in mirrored position-block pairs (fwd consumes ascending, bwd
descending; blocks stay resident so each is computed once).

Per superstep: fwd chains are matmul + DVE drain (PSUM * fx).  bwd
chains use two forms to balance engines around the hw rule that only
DVE/Activation may touch PSUM: a few "direct" columns per quad mirror
the fwd form on DVE; the rest run in pre-multiply form (Pool computes
fx*u in SBUF, PE applies X^T, Activation/DVE copy PSUM back to SBUF).
On the last superstep the split columns' PSUM holds b_s directly.
"""

import sys

sys.path.insert(0, "/opt/trn_rl_repo")

import numpy as np
import ml_dtypes

B, L, T = 1024, 1024, 32
START, STOP = 30, 31
NCORES = 8
BS = B // NCORES          # sequences per core
K = 32                    # steps per segment == supersteps
S = L // K                # segments
NQ = 8                    # quads (4 segments each)
MU = 3.88                 # per-step log-growth bias
DK = 2                    # k-steps per position block
NBLK = K // DK            # position blocks (16)
NWIN = NBLK // 2          # mirrored load windows (8)
CS = 12                   # direct bwd cols per quad ([0:CS)); rest split-form
SPL = 128 - CS            # split cols per quad
CA1 = 31                  # first half: Act copies quad-cols [CS:CA1)
CA2 = 128                 # second half: Act copies all split cols

_compiled = None


def _build_nc():
    import concourse.bacc as bacc
    import concourse.tile as tile
    import concourse.mybir as mybir
    import concourse.masks as masks

    fp32 = mybir.dt.float32
    bf16 = mybir.dt.bfloat16
    Exp = mybir.ActivationFunctionType.Exp
    Ln = mybir.ActivationFunctionType.Ln
    Copy = mybir.ActivationFunctionType.Copy
    mult = mybir.AluOpType.mult

    nc = bacc.Bacc(
        "TRN2",
        target_bir_lowering=False,
        debug=False,
        enable_asserts=False,
        num_devices=NCORES,
    )
    staged_d = nc.dram_tensor("staged", [128, K * NQ * 128], bf16, kind="ExternalInput").ap()
    gvals_d = nc.dram_tensor("gvals", [128, L], bf16, kind="ExternalInput").ap()
    counts_d = nc.dram_tensor("counts", [128, 8 * 128], fp32, kind="ExternalInput").ap()
    trans_d = nc.dram_tensor("trans", [T, T], fp32, kind="ExternalInput").ap()
    out_d = nc.dram_tensor("out", [1, BS], fp32, kind="ExternalOutput").ap()

    from contextlib import ExitStack

    with tile.TileContext(nc) as tc, ExitStack() as ctx:
        singles = ctx.enter_context(tc.tile_pool(name="singles", bufs=1))
        stg_pool = ctx.enter_context(tc.tile_pool(name="stg", bufs=6))
        fx_pool = ctx.enter_context(tc.tile_pool(name="fx", bufs=16))
        st_pool = ctx.enter_context(tc.tile_pool(name="st", bufs=2))
        ps_pool = ctx.enter_context(tc.tile_pool(name="ps", bufs=1, space="PSUM"))
        sm_pool = ctx.enter_context(tc.tile_pool(name="sm", bufs=2))

        # ---- input DMAs ------------------------------------------------
        # trans replicas first on the HWDGE queue (needed to build the W
        # matrices before anything else), then the 16 shared position
        # blocks in mirrored pair order.  counts/gvals/trans-flat ride the
        # SWDGE (gpsimd) queue; they are consumed mid-loop.
        trans_rep = singles.tile([128, T], fp32, tag="trans_rep")
        for kk in range(4):
            nc.sync.dma_start(out=trans_rep[32 * kk : 32 * (kk + 1), :], in_=trans_d)
        stg = [None] * NBLK
        for j in range(NWIN):
            for m in (j, NBLK - 1 - j):
                t = stg_pool.tile([128, DK * 1024], bf16, tag="stg", name=f"stg_{m}")
                nc.sync.dma_start(
                    out=t[:], in_=staged_d[:, 2 * m * 1024 : (2 * m + 2) * 1024]
                )
                stg[m] = t

        # ones4[32a+t, a] = 1 (partition-sum per slice)
        ones4 = singles.tile([128, 4], bf16, tag="ones4")
        nc.gpsimd.memset(ones4[:], 0.0)
        for a in range(4):
            nc.gpsimd.memset(ones4[32 * a : 32 * a + 32, a : a + 1], 1.0)
        ones41 = singles.tile([4, 1], bf16, tag="ones41")
        nc.gpsimd.memset(ones41[:], 1.0)

        identf = singles.tile([128, 128], fp32, tag="identf")
        masks.make_identity(nc, identf[:])

        # ---- state init ------------------------------------------------
        st_f = st_pool.tile([128, 1024], bf16, tag="stf", name="stf_init")
        nc.vector.memset(st_f[:], 1.0)
        nc.vector.memset(st_f[0:32, 0:128], 0.0)
        nc.gpsimd.affine_select(
            out=st_f[0:32, 0:128], in_=st_f[0:32, 0:128], pattern=[[0, 128]],
            compare_op=mybir.AluOpType.not_equal, fill=1.0,
            base=-START, channel_multiplier=1,
        )

        counts_sb = singles.tile([128, 8 * 128], fp32, tag="counts_sb")
        nc.gpsimd.dma_start(out=counts_sb[:], in_=counts_d)
        g_sb = singles.tile([128, L], bf16, tag="g_sb")
        nc.gpsimd.dma_start(out=g_sb[:], in_=gvals_d)
        # rhs for the count matmuls: rhs_tc[p, c] = trans.flat[c*128 + p]
        rhs_tc = singles.tile([128, 8], fp32, tag="rhs_tc")
        tflat = trans_d.rearrange("a (b o) -> (a b) o", o=1)
        for c in range(8):
            nc.gpsimd.dma_start(out=rhs_tc[:, c : c + 1], in_=tflat[c * 128 : (c + 1) * 128, :])

        # ---- constants -------------------------------------------------
        # X_rep[32k+i, j] = X[i, j] = exp(trans[i, j])
        x_rep = singles.tile([128, T], bf16, tag="x_rep")
        nc.scalar.activation(x_rep[:], trans_rep[:], Exp)
        # Xt_rep[32k+j, i] = X[i, j]
        xt_rep = singles.tile([128, T], bf16, tag="xt_rep")
        nc.vector.transpose(xt_rep[:], x_rep[:])

        # W_f: blockdiag lhsT for out = X @ in  (lhsT[j,i] = X[i,j])
        w_f = singles.tile([128, 128], bf16, tag="w_f")
        nc.vector.memset(w_f[:], 0.0)
        for a in range(4):
            nc.vector.tensor_copy(w_f[32 * a : 32 * a + 32, 32 * a : 32 * a + 32],
                                  xt_rep[32 * a : 32 * a + 32, :])
        # W_b: blockdiag lhsT for out = X^T @ in  (lhsT[j,i] = X[j,i])
        w_b = singles.tile([128, 128], bf16, tag="w_b")
        nc.vector.memset(w_b[:], 0.0)
        for a in range(4):
            nc.vector.tensor_copy(w_b[32 * a : 32 * a + 32, 32 * a : 32 * a + 32],
                                  x_rep[32 * a : 32 * a + 32, :])

        bias = singles.tile([128, 1], fp32, tag="bias")
        nc.vector.memset(bias[:], -MU)

        # stopcol[32k+j] = X[STOP, j] (fp32 for tensor_scalar)
        stopcol = singles.tile([128, 1], fp32, tag="stopcol")
        nc.vector.tensor_copy(stopcol[:], xt_rep[:, STOP : STOP + 1])

        # persistent psum for the gold/count results
        psg = ps_pool.tile([128, 512], fp32, tag="psg")
        emit = sm_pool.tile([128, 1], fp32, tag="emit")

        # ---- exp blocks ------------------------------------------------
        fx = [None] * NBLK

        def emit_exp_pair(j):
            for m in (j, NBLK - 1 - j):
                t = fx_pool.tile([128, DK * 1024], bf16, tag="fx", name=f"fx_{m}")
                nc.scalar.activation(t[:], stg[m][:], Exp, bias=bias[:])
                fx[m] = t

        emit_exp_pair(0)

        def q3(ap, lo, hi, width):
            # [p, (q c)] view restricted to per-quad cols [lo:hi)
            return ap.rearrange("p (q c) -> p q c", c=width)[:, :, lo:hi]

        # bwd init (phase 1 is all-direct, full width):
        # w_0 = d_31 * q  (q = ones; stopvec on slice (7,3))
        st_b = st_pool.tile([128, 1024], bf16, tag="stb", name="stb_init")
        nc.vector.tensor_copy(st_b[:], fx[15][:, 1024:2048])
        nc.vector.tensor_scalar(
            st_b[96:128, 896:1024], fx[15][96:128, 1024 + 896 : 1024 + 1024],
            stopcol[96:128, :], None, mult,
        )

        mega_f = ps_pool.tile([128, 1024], fp32, tag="mega_f")
        mega_b = ps_pool.tile([128, 1024], fp32, tag="mega_b")

        # u-form sub-streams for phase 2 (per-quad col ranges).  Each
        # stream owns its psum/t/u tiles so the streams pipeline
        # independently: premult (DVE, idle once fwd drains shrink) ->
        # X^T matmuls -> Act copy -> next premult.
        STREAMS = [(CS, 70), (70, 128)]
        SW = [hi - lo for lo, hi in STREAMS]
        ps_str = [
            ps_pool.tile([128, NQ * w], fp32, tag=f"pstr{i}", name=f"pstr{i}")
            for i, w in enumerate(SW)
        ]
        ps_dir = ps_pool.tile([128, NQ * CS], fp32, tag="psdir")
        u_st = [None] * len(STREAMS)
        stc = None  # compact direct-col state [128, NQ*CS] in phase 2

        def qq(ap, w):
            return ap.rearrange("p (q c) -> p q c", c=w)

        # ---- main loop -------------------------------------------------
        for k in range(K):
            pos = K - 1 - k
            fxf, off_f = fx[k // DK], (k % DK) * 1024
            if k % DK == 0 and k // DK + 1 < NWIN:
                emit_exp_pair(k // DK + 1)
            if k == 20:
                # gold emission sum; DVE has phase-2 slack
                nc.vector.tensor_reduce(
                    emit[:], g_sb[:].rearrange("p (o l) -> p o l", o=1),
                    axis=mybir.AxisListType.X, op=mybir.AluOpType.add,
                )
            if k == 22:
                # gold transition score: 8 accumulating matmuls (PE slack)
                for c in range(8):
                    nc.tensor.matmul(
                        psg[0:1, 0:128], rhs_tc[:, c : c + 1],
                        counts_sb[:, c * 128 : (c + 1) * 128],
                        start=(c == 0), stop=(c == 7),
                    )
                nc.tensor.matmul(psg[0:1, 128:256], emit[:], identf[:],
                                 is_transpose=True)
            # fwd
            for q in range(NQ):
                nc.tensor.matmul(
                    mega_f[:, q * 128 : (q + 1) * 128], w_f,
                    st_f[:, q * 128 : (q + 1) * 128], start=True, stop=True,
                )
            nst_f = st_pool.tile([128, 1024], bf16, tag="stf", name=f"stf_{k}")
            nc.vector.tensor_mul(nst_f[:], mega_f[:], fxf[:, off_f : off_f + 1024])
            st_f = nst_f

            if k < 16:
                # phase 1 bwd: full-width direct (matmul + DVE drain)
                fxb, off_b = fx[pos // DK], (pos % DK) * 1024
                if k > 0:
                    for q in range(NQ):
                        nc.tensor.matmul(
                            mega_b[:, q * 128 : (q + 1) * 128], w_b,
                            st_b[:, q * 128 : (q + 1) * 128],
                            start=True, stop=True,
                        )
                    nst_b = st_pool.tile([128, 1024], bf16, tag="stb",
                                         name=f"stb_{k}")
                    nc.vector.tensor_mul(
                        nst_b[:], mega_b[:], fxb[:, off_b : off_b + 1024])
                    st_b = nst_b
            else:
                # phase 2 bwd.  Direct cols [0:CS) stay in drain form on DVE.
                # Split cols run in pre-multiply form; the premult applies
                # position 32-k (one behind); k=16 converts (matmul only).
                for i, (lo, hi) in enumerate(STREAMS):
                    w = SW[i]
                    if k > 16:
                        pp = 32 - k
                        fxb, off_b = fx[pp // DK], (pp % DK) * 1024
                        tmi = st_pool.tile([128, NQ * w], bf16, tag=f"tm{i}",
                                           name=f"t{i}_{k}")
                        nc.vector.tensor_mul(
                            qq(tmi[:], w), qq(u_st[i][:], w),
                            qq(fxb[:, off_b : off_b + 1024], 128)[:, :, lo:hi],
                        )
                        rhs_t = tmi
                    else:
                        rhs_t = None
                    for q in range(NQ):
                        rhs = (st_b[:, q * 128 + lo : q * 128 + hi]
                               if k == 16 else
                               rhs_t[:, q * w : (q + 1) * w])
                        nc.tensor.matmul(
                            ps_str[i][:, q * w : (q + 1) * w], w_b, rhs,
                            start=True, stop=True,
                        )
                    nu = st_pool.tile([128, NQ * w], bf16, tag=f"ust{i}",
                                      name=f"u{i}_{k}")
                    nc.scalar.activation(nu[:], ps_str[i][:], Copy)
                    u_st[i] = nu
                # direct cols
                fxb2, off_b2 = fx[pos // DK], (pos % DK) * 1024
                for q in range(NQ):
                    rhs = (st_b[:, q * 128 : q * 128 + CS] if k == 16
                           else stc[:, q * CS : (q + 1) * CS])
                    nc.tensor.matmul(
                        ps_dir[:, q * CS : (q + 1) * CS], w_b, rhs,
                        start=True, stop=True,
                    )
                nstc = st_pool.tile([128, NQ * CS], bf16, tag="stc",
                                    name=f"stc_{k}")
                nc.vector.tensor_mul(
                    qq(nstc[:], CS), qq(ps_dir[:], CS),
                    qq(fxb2[:, off_b2 : off_b2 + 1024], 128)[:, :, 0:CS],
                )
                stc = nstc

        # tail of the bwd chains: position 0 premult + matmul; the stream
        # psums then hold b_s
        tm_tail = []
        for i, (lo, hi) in enumerate(STREAMS):
            w = SW[i]
            tmi = st_pool.tile([128, NQ * w], bf16, tag=f"tm{i}",
                               name=f"t{i}_tail")
            nc.vector.tensor_mul(
                qq(tmi[:], w), qq(u_st[i][:], w),
                qq(fx[0][:, 0:1024], 128)[:, :, lo:hi],
            )
            tm_tail.append(tmi)
        st_b = stc  # direct-col final state (w_s), compact layout

        # ---- join ------------------------------------------------------
        # yx = X a_{s-1} (for direct cols' junctions), shifted one slice up;
        # yi = a_{s-1} (for split cols, whose psum already holds b_s).
        # Both computed full-width in mega_f then copied to SBUF.
        def shift_mms(lhs_rep, dest):
            for c0 in (0, 512):
                for a in range(1, 4):
                    nc.tensor.matmul(
                        dest[32 * a : 32 * a + 32, c0 : c0 + 512],
                        lhs_rep[32 * (a - 1) : 32 * a, :],
                        st_f[32 * (a - 1) : 32 * a, c0 : c0 + 512],
                        start=True, stop=True,
                        tile_position=(32 * (a - 1), 32 * a),
                    )
            for c0 in (128, 640):
                nc.tensor.matmul(
                    dest[0:32, c0 : c0 + 384], lhs_rep[96:128, :],
                    st_f[96:128, c0 - 128 : c0 + 256],
                    start=True, stop=True, tile_position=(96, 0),
                )
            nc.tensor.matmul(
                dest[0:32, 512 : 512 + 128], lhs_rep[96:128, :],
                st_f[96:128, 384:512], start=True, stop=True,
                tile_position=(96, 0),
            )
            nc.tensor.matmul(
                dest[0:32, 0:128], lhs_rep[96:128, :], st_f[96:128, 896:1024],
                start=True, stop=True, tile_position=(96, 0),
            )

        shift_mms(xt_rep, mega_f)
        yx = sm_pool.tile([128, 1024], bf16, tag="yx")
        nc.scalar.activation(yx[:], mega_f[:], Copy)
        # preload the Ln act table while z/Jp run (ln of 1.0 into scratch)
        lnwarm = sm_pool.tile([1, 1], fp32, tag="lnwarm")
        nc.scalar.activation(lnwarm[:], identf[0:1, 0:1], Ln)

        # z = b_s * (shifted a): direct cols from st_b, split cols from
        # the per-stream psums
        z_sb = sm_pool.tile([128, 1024], bf16, tag="z_sb")
        nc.vector.tensor_mul(
            q3(z_sb[:], 0, CS, 128), q3(st_b[:], 0, CS, CS), q3(yx[:], 0, CS, 128))
        for i, (lo, hi) in enumerate(STREAMS):
            w = SW[i]
            nc.vector.tensor_mul(
                q3(z_sb[:], lo, hi, 128), qq(tm_tail[i][:], w),
                q3(yx[:], lo, hi, 128))
        # exclusion s=S-1 for the normalizer: overwrite a_{S-1} (no longer
        # needed) with 1/32 so its column sum is 1 and ln is 0
        nc.vector.memset(st_f[96:128, 896:1024], 1.0 / 32.0)
        # junction and normalizer partition-sums
        for c0 in (0, 512):
            nc.tensor.matmul(
                mega_f[0:4, c0 : c0 + 512], ones4, z_sb[:, c0 : c0 + 512],
                start=True, stop=True,
            )
            nc.tensor.matmul(
                mega_f[32:36, c0 : c0 + 512], ones4, st_f[:, c0 : c0 + 512],
                start=True, stop=True, tile_position=(0, 32),
            )
        # ln -> diff -> sum, split by column halves so Act/DVE/PE overlap
        lnj = sm_pool.tile([4, 1024], fp32, tag="lnj")
        lnc = sm_pool.tile([4, 1024], fp32, tag="lnc")
        diff = sm_pool.tile([4, 1024], bf16, tag="diff")
        for c0 in (0, 512):
            nc.scalar.activation(lnj[:, c0 : c0 + 512],
                                 mega_f[0:4, c0 : c0 + 512], Ln)
            nc.scalar.activation(lnc[:, c0 : c0 + 512],
                                 mega_f[32:36, c0 : c0 + 512], Ln)
            nc.vector.tensor_sub(diff[:, c0 : c0 + 512],
                                 lnj[:, c0 : c0 + 512], lnc[:, c0 : c0 + 512])
            if c0 == 0:
                nc.vector.memset(diff[0:1, 0:128], 0.0)
        # logZ~ (per seq) = sum over (a, q): 8 accumulating matmuls
        for q in range(NQ):
            nc.tensor.matmul(
                psg[0:1, 256:384], ones41, diff[:, q * 128 : (q + 1) * 128],
                start=(q == 0), stop=(q == 7),
            )
        # nll = logZ~ + L*mu - emit - transpart  (one PSUM operand per op)
        emt = sm_pool.tile([1, 128], fp32, tag="emt")
        nc.vector.tensor_copy(emt[:], psg[0:1, 128:256])
        t1 = sm_pool.tile([1, 128], fp32, tag="t1")
        nc.vector.scalar_tensor_tensor(
            t1[:], psg[0:1, 256:384], 1.0, emt[:],
            op0=mult, op1=mybir.AluOpType.subtract)
        res = sm_pool.tile([1, 128], fp32, tag="res")
        nc.vector.scalar_tensor_tensor(
            res[:], t1[:], float(L) * MU, psg[0:1, 0:128],
            op0=mybir.AluOpType.add, op1=mybir.AluOpType.subtract)
        nc.sync.dma_start(out=out_d, in_=res[:])

    nc.compile()
    return nc


def _stage_core(feats_c, tags_c):
    """feats_c [128, 1024, 32] f32, tags_c [128, 1024] int -> dict of arrays."""
    bf16 = ml_dtypes.bfloat16
    # staged[32a+t, k, q, b] = feats_c[b, q*128 + a*32 + k, t]
    f = np.ascontiguousarray(feats_c.transpose(1, 2, 0))  # [l, t, b]
    f = f.reshape(NQ, 4, K, T, BS)                        # [q, a, k, t, b]
    staged = np.ascontiguousarray(f.transpose(1, 3, 2, 0, 4)).reshape(128, K * NQ * BS)
    # gathered emission values
    g = np.take_along_axis(feats_c, tags_c[:, :, None].astype(np.int64), axis=2)[:, :, 0]
    # transition pair counts: pair = to*32 + from over (START+tags, tags+STOP)
    pad_start = np.concatenate(
        [np.full((BS, 1), START, tags_c.dtype), tags_c], axis=1)
    pad_stop = np.concatenate(
        [tags_c, np.full((BS, 1), STOP, tags_c.dtype)], axis=1)
    pair = (pad_stop.astype(np.int64) * T + pad_start.astype(np.int64))  # [BS, L+1]
    cnt = np.zeros((BS, T * T), np.float32)
    np.add.at(cnt, (np.arange(BS)[:, None], pair), 1.0)
    counts = np.ascontiguousarray(
        cnt.T.reshape(8, 128, BS).transpose(1, 0, 2)).reshape(128, 8 * BS)
    return {
        "staged": staged.astype(bf16),
        "gvals": g.astype(bf16),
        "counts": counts,
    }


LAST_RESULTS = None


def kernel(feats, transitions, tags, _trace=False):
    global _compiled, LAST_RESULTS
    from concourse.bass_utils import run_bass_kernel_spmd

    feats = np.asarray(feats, dtype=np.float32)
    transitions = np.asarray(transitions, dtype=np.float32)
    tags = np.asarray(tags)

    if _compiled is None:
        _compiled = _build_nc()
    nc = _compiled

    in_maps = []
    for c in range(NCORES):
        sl = slice(c * BS, (c + 1) * BS)
        m = _stage_core(feats[sl], tags[sl])
        m["trans"] = transitions
        in_maps.append(m)
    res = run_bass_kernel_spmd(
        nc, in_maps, core_ids=list(range(NCORES)), trace=_trace
    )
    LAST_RESULTS = res
    out = np.concatenate([r["out"].reshape(BS) for r in res.results])
    return out.astype(np.float32)
